# revision 47
# baseline (speedup 1.0000x reference)
"""Binary complex conv (BC conv) on 8 TRN2 NeuronCores.

Reference computation:
    xb = sign(x + 1e-6)                      # (16, 256, 112, 112)
    wr = sign(weight_real + 1e-6)            # (128, 128, 3, 3)
    wi = sign(weight_imag + 1e-6)
    kernel = [[wr, -wi], [wi, wr]]           # (256, 256, 3, 3)
    out = conv2d(xb, kernel, pad=1) + bias   # (16, 256, 112, 112)

Strategy: pure data-parallel over batch (2 images per core); everything
else on-device, numerically exact in the matmuls (all operands are
+-1/0/+-2 -> exact in fp8e4; PSUM accumulates fp32; x is uploaded and
out downloaded as bf16, within the 2e-2 gate by a wide margin).

Tricks on top of the direct conv:
 * Karatsuba for the complex structure: A = xr*wr, B = xi*wi,
   C = (xr+xi)*(wr+wi); out_real = A-B, out_imag = C-A-B.
   3 convs of 128 input channels instead of 4.
 * fp8 DoubleRow: each binarized frame is stored with row stride 114;
   conv taps in raster order have flat offsets [-115,-114,-113,-1,0,1,
   113,114,115], so consecutive taps pair into DoubleRow matmuls
   (contraction 256) with pair strides 1/112/1/1 + one normal matmul.
 * 448-wide matmuls: the moving operand walks [pair, row(x114), col(x112)]
   so the 2 pad columns per frame row are never streamed and the
   4-output-row PSUM bank is compact [128, 4, 112].
 * bf16 I/O halves HBM traffic (52.6 -> 26.9 MB/core): no output-DMA
   backlog at the tail, no DMA-contention stalls mid-stream.

Schedule (measured on HW): PE streams 840 matmuls back-to-back at
1 col/cycle with LDWEIGHTS hidden (~164us core at 2.36GHz, zero stalls).
Head ~15us: engine preamble (~7us, fixed) + junk-MM HAM warmup sized to
end exactly at data-ready; the Sign ACT_TABLE_LOAD is hoisted by a dummy
activation; wr/strips ride separate DMA rings.  Strips are binarized
JUST IN TIME between conv tile-pairs -- ScalarE's FIFO must never hold
long Sign ops ahead of pending PSUM evacuations or the banks clog and
the PE stalls.  Tail ~6us: last tile runs its s-conv first and splits
evac/combine/DMA into 2-row halves.
"""

import numpy as np

import concourse.bass as bass
import concourse.tile as tile
from concourse import mybir
from concourse.bass_utils import run_bass_kernel_spmd

N_CORES = 8
B = 16
CPB = 128          # channels per block (partition dim)
H = W = 112
RS = 114           # frame row stride
FROWS = 116        # 114 padded rows + 2 junk margin rows
IMGS = 2
TROWS = 4          # output rows per matmul tile
NT = TROWS * RS    # matmul free dim (456)
NTILES = H // TROWS
BAND = 28          # input rows binarized per activation op
EPS = 1e-6

F32 = mybir.dt.float32
BF16 = mybir.dt.bfloat16
FP8 = mybir.dt.float8e4
AF = mybir.ActivationFunctionType
DRM = mybir.MatmulPerfMode.DoubleRow
ALU = mybir.AluOpType

# tap flat offsets in raster order; pairs (0,1) (2,3) (4,5) (6,7), single 8
TAP_OFF = [dy * RS + dx for dy in (-1, 0, 1) for dx in (-1, 0, 1)]


def _split_multiwait(nc):
    """Walrus in this container rejects >1 semaphore wait per instruction
    ("Too many sync wait commands"); hoist extra waits onto preceding nops
    on the same engine."""
    import bass_rust

    for f in nc.m.functions:
        for bb in f.blocks:
            new_insts = []
            for inst in bb.instructions:
                si = inst.sync_info
                waits = list(si.on_wait) if si is not None and si.on_wait else []
                if len(waits) > 1:
                    for w in waits[:-1]:
                        nop = mybir.InstNoOp(
                            name=nc.get_next_instruction_name(),
                            engine=inst.engine,
                            ins=[],
                            outs=[],
                        )
                        nop.sync_info = bass_rust.SyncInfo(on_wait=[w], on_update=[])
                        new_insts.append(nop)
                    si.on_wait = [waits[-1]]
                    inst.sync_info = si
                new_insts.append(inst)
            bb.instructions = new_insts


def build_nc():
    nc = bass.Bass()

    x_ext = nc.declare_dram_parameter("x", [IMGS, 2 * CPB, H, W], BF16, isOutput=False)
    wr_ext = nc.declare_dram_parameter("wrT", [CPB, 9 * CPB], BF16, isOutput=False)
    wi_ext = nc.declare_dram_parameter("wiT", [CPB, 9 * CPB], BF16, isOutput=False)
    bias_ext = nc.declare_dram_parameter("bias2", [CPB, 2], F32, isOutput=False)
    out_ext = nc.declare_dram_parameter("out", [IMGS, 2 * CPB, H, W], BF16, isOutput=True)

    x_flat = x_ext.rearrange("b c h w -> (b c) h w")       # [512, 112, 112]
    out_flat = out_ext.rearrange("b c h w -> (b c) h w")

    with tile.TileContext(nc) as tc:
        with (
            tc.tile_pool(name="wstage", bufs=2) as wstage_pool,
            tc.tile_pool(name="wbin", bufs=1) as wbin_pool,
            tc.tile_pool(name="biasp", bufs=1) as bias_pool,
            tc.tile_pool(name="xq", bufs=1) as xq_pool,
            tc.tile_pool(name="stage", bufs=4) as stage_pool,
            tc.tile_pool(name="tmp", bufs=6) as tmp_pool,
            tc.tile_pool(name="outsb", bufs=8) as out_pool,
            tc.tile_pool(name="psum", bufs=8, space="PSUM") as psum_pool,
        ):
            # HAM warmup: dummy matmuls on junk data with no dependencies so
            # the PE clock-gate reaches 8/8 before the first real matmul.
            # The memset is VectorE's very first op; sized to end right as
            # the first binarized strip + weights land.
            junk = bias_pool.tile([CPB, 512], FP8, tag="junk")
            nc.vector.memset(junk[:], 1.0)
            jps = psum_pool.tile([CPB, 512], F32, tag="ps", name="jps")
            for _ in range(14):
                nc.tensor.matmul(jps[:], junk[:, :CPB], junk[:], start=True,
                                 stop=True)

            # per-partition scalar constant for activation bias
            eps_pos = bias_pool.tile([CPB, 1], F32, tag="epsp")
            nc.vector.memset(eps_pos[:], EPS)

            # dummy Sign on ScalarE: hoists the ~1.3us ACT_TABLE_LOAD off
            # the first real sign's critical path
            tbl = bias_pool.tile([CPB, 1], FP8, tag="tbl")
            nc.scalar.activation(tbl[:], eps_pos[:], AF.Sign, bias=eps_pos[:],
                                 scale=1.0)

            # ---- weights: wr first on Sync's ring (wr -> wq_r sign gates
            # the first conv); the head x strips ride ScalarE's ring ----
            wr_f32 = wstage_pool.tile([CPB, 9 * CPB], BF16, tag="wstage")
            nc.sync.dma_start(wr_f32[:], wr_ext[:])
            wi_f32 = wstage_pool.tile([CPB, 9 * CPB], BF16, tag="wstage")
            nc.sync.dma_start(wi_f32[:, :576], wi_ext[:, :576])
            nc.sync.dma_start(wi_f32[:, 576:], wi_ext[:, 576:])

            # binarized fp8 weights [ci, tap, co]; wq_s = wq_r + wq_i
            wq_r = wbin_pool.tile([CPB, 9, CPB], FP8, tag="wqr")
            wq_i = wbin_pool.tile([CPB, 9, CPB], FP8, tag="wqi")
            wq_s = wbin_pool.tile([CPB, 9, CPB], FP8, tag="wqs")
            wr_v = wr_f32[:].rearrange("p (t c) -> p t c", c=CPB)
            wi_v = wi_f32[:].rearrange("p (t c) -> p t c", c=CPB)

            bias_sb = bias_pool.tile([CPB, 2], F32)
            nc.gpsimd.dma_start(bias_sb[:], bias_ext[:])
            bias_ir = bias_pool.tile([CPB, 1], F32, tag="biasir")
            nc.vector.tensor_add(bias_ir[:], bias_sb[:, 1:2], bias_sb[:, 0:1])

            # ---- persistent binarized fp8 frames ----
            # frame: [128, FROWS, RS]; frame row = padded row + 1 (1 junk
            # margin row on top); cols 0 / 113 are the zero pad columns,
            # cols 114-115 slack (only ever read into discarded pad lanes)
            def frame(nm):
                return xq_pool.tile([CPB, FROWS, RS], FP8, tag=nm, name=nm)

            xqr = [frame(f"xqr{i}") for i in range(IMGS)]
            xqi = [frame(f"xqi{i}") for i in range(IMGS)]
            xqs = [frame(f"xqs{i}") for i in range(IMGS)]
            for i in range(IMGS):
                eng = nc.vector if i == 0 else nc.gpsimd
                for t in (xqr[i], xqi[i], xqs[i]):
                    eng.memset(t[:, 1:2, :], 0.0)          # padded row 0
                    eng.memset(t[:, 114:115, :], 0.0)      # padded row 113
                    eng.memset(t[:, 1:115, 0:1], 0.0)      # padded col 0
                    eng.memset(t[:, 1:115, 113:114], 0.0)  # padded col 113

            flat = {}
            for i in range(IMGS):
                flat[("r", i)] = xqr[i][:].rearrange("p r c -> p (r c)")
                flat[("i", i)] = xqi[i][:].rearrange("p r c -> p (r c)")
                flat[("s", i)] = xqs[i][:].rearrange("p r c -> p (r c)")

            # ---- binarize input + build the sum frame, band by band ----
            def stage_dma(img, cib, r0, nr, dma_eng=None):
                ch0 = img * 2 * CPB + cib * CPB
                st = stage_pool.tile([CPB, BAND, W], BF16, tag="stage")
                (dma_eng or nc.sync).dma_start(
                    st[:, :nr, :],
                    x_flat[ch0:ch0 + CPB, r0:r0 + nr, :],
                )
                return st

            def sign_stage(img, cib, r0, nr, st):
                rows = slice(r0 + 2, r0 + 2 + nr)
                dst = xqr if cib == 0 else xqi
                nc.scalar.activation(
                    dst[img][:, rows, 1:113], st[:, :nr, :],
                    AF.Sign, bias=eps_pos[:], scale=1.0,
                )

            def stage_sign(img, cib, r0, nr, dma_eng=None):
                sign_stage(img, cib, r0, nr, stage_dma(img, cib, r0, nr, dma_eng))

            def sum_rows(img, r0, nr):
                rows = slice(r0 + 2, r0 + 2 + nr)
                nc.vector.tensor_tensor(
                    xqs[img][:, rows, 1:113],
                    xqr[img][:, rows, 1:113],
                    xqi[img][:, rows, 1:113],
                    op=ALU.add,
                )

            def binarize_rows(img, r0, nr):
                stage_sign(img, 0, r0, nr)
                stage_sign(img, 1, r0, nr)
                sum_rows(img, r0, nr)

            def binarize_band(img, b, strips=1):
                r0 = b * BAND
                step = BAND // strips
                for s in range(strips):
                    binarize_rows(img, r0 + s * step, step)

            def conv_one(img, t, kind):
                # free dim walks [row, col] with the frame's 2 pad columns
                # skipped: 448 columns per matmul instead of 456 (-1.75%
                # PE time), and the PSUM bank comes out compact
                base = (4 * t + 2) * RS + 1
                w3 = {"r": wq_r, "i": wq_i, "s": wq_s}[kind]
                xf = flat[(kind, img)]
                ps = psum_pool.tile([CPB, TROWS, W], F32, tag="ps",
                                    name=f"ps_{kind}{img}_{t}")
                part = [list(xf.ap)[0][0], CPB]
                for p in range(4):
                    o0, o1 = TAP_OFF[2 * p], TAP_OFF[2 * p + 1]
                    rhs = bass.AP(
                        xf.tensor, xf.offset + o0 + base,
                        [part, [o1 - o0, 2], [RS, TROWS], [1, W]],
                    )
                    nc.tensor.matmul(
                        ps[:], w3[:, 2 * p:2 * p + 2, :], rhs,
                        start=(p == 0), stop=False, perf_mode=DRM,
                    )
                rhs8 = bass.AP(
                    xf.tensor, xf.offset + TAP_OFF[8] + base,
                    [part, [RS, TROWS], [1, W]],
                )
                nc.tensor.matmul(ps[:], w3[:, 8, :], rhs8,
                                 start=False, stop=True)
                return ps

            # out_real = A - B + bias_r ; out_imag = C - A - B + bias_i
            # ScalarE (fast PSUM port) evacuates each bank compactly
            # right after its conv, exactly one reader per bank:
            #   An2 = A + bias_r ; Bn0 = -B
            # then out_real = An2 + Bn0 (SBUF-only, GpSimd)
            #      out_imag = ((C - An2) + (bias_i+bias_r)) + Bn0
            def evac_A(A):
                An2 = tmp_pool.tile([CPB, TROWS, W], F32, tag="An")
                nc.scalar.activation(An2[:], A[:], AF.Identity,
                                     bias=bias_sb[:, 0:1], scale=1.0)
                return An2

            def evac_B(Bp):
                Bn0 = tmp_pool.tile([CPB, TROWS, W], F32, tag="Bn")
                nc.scalar.activation(Bn0[:], Bp[:], AF.Identity,
                                     bias=0.0, scale=-1.0)
                return Bn0

            def combine(img, t, An2, Bn0, C, re_eng=None):
                osb = out_pool.tile([CPB, 2, TROWS, W], BF16, tag="osb")
                t5 = tmp_pool.tile([CPB, TROWS, W], F32, tag="t5")
                nc.vector.tensor_sub(t5[:], C[:], An2[:])
                (re_eng or nc.gpsimd).tensor_tensor(
                    osb[:, 0], An2[:], Bn0[:], op=ALU.add)
                nc.vector.scalar_tensor_tensor(
                    osb[:, 1], t5[:], bias_ir[:], Bn0[:],
                    op0=ALU.add, op1=ALU.add,
                )

                # one DMA for both channel halves: dst walks [ch-within-
                # block, block, row, col] to match the tile's layout
                dst = bass.AP(
                    out_flat.tensor,
                    img * 2 * CPB * H * W + 4 * t * W,
                    [[H * W, CPB], [CPB * H * W, 2], [W, TROWS], [1, W]],
                )
                nc.sync.dma_start(dst, osb[:])

            def combine_half(img, t, An2, Bn0h, C, h):
                # 2-row half of the last tile: short serial chain after the
                # final matmul, so the closing DMA completes sooner
                rows = slice(2 * h, 2 * h + 2)
                osb = out_pool.tile([CPB, 2, 2, W], BF16, tag="osbh")
                t5 = tmp_pool.tile([CPB, 2, W], F32, tag="t5h")
                nc.vector.tensor_sub(t5[:], C[:, rows], An2[:, rows])
                nc.vector.tensor_tensor(
                    osb[:, 0], An2[:, rows], Bn0h[:], op=ALU.add)
                nc.vector.scalar_tensor_tensor(
                    osb[:, 1], t5[:], bias_ir[:], Bn0h[:],
                    op0=ALU.add, op1=ALU.add,
                )
                dst = bass.AP(
                    out_flat.tensor,
                    img * 2 * CPB * H * W + (4 * t + 2 * h) * W,
                    [[H * W, CPB], [CPB * H * W, 2], [W, 2], [1, W]],
                )
                # halves go out on different rings so their ~0.65us issue
                # slices run in parallel at the very end
                (nc.scalar if h else nc.sync).dma_start(dst, osb[:])

            def evac_B_half(Bp, h):
                Bn0 = tmp_pool.tile([CPB, 2, W], F32, tag="Bnh")
                nc.scalar.activation(Bn0[:], Bp[:, 2 * h:2 * h + 2],
                                     AF.Identity, bias=0.0, scale=-1.0)
                return Bn0

            def conv_tiles(img, tiles, last=False):
                for t in tiles:
                    if last and t == tiles[-1]:
                        # last tile: s-conv first so t5 is off the critical
                        # path; evac+combine+DMA split into 2-row halves
                        C = conv_one(img, t, "s")
                        An2 = evac_A(conv_one(img, t, "r"))
                        Bp = conv_one(img, t, "i")
                        b0 = evac_B_half(Bp, 0)
                        combine_half(img, t, An2, b0, C, 0)
                        b1 = evac_B_half(Bp, 1)
                        combine_half(img, t, An2, b1, C, 1)
                        continue
                    An2 = evac_A(conv_one(img, t, "r"))
                    Bn0 = evac_B(conv_one(img, t, "i"))
                    C = conv_one(img, t, "s")
                    combine(img, t, An2, Bn0, C)

            def conv_tiles_kindmajor(img, tiles, mid=None):
                # first tiles: group by kind so the i/s convs' inputs
                # (wq_i, wq_s, xqs) get extra time to become ready
                As = [(t, conv_one(img, t, "r")) for t in tiles]
                Ans = [(t, evac_A(A)) for t, A in As]
                if mid:
                    mid()
                Bs = [(t, conv_one(img, t, "i")) for t in tiles]
                Bns = [(t, evac_B(B)) for t, B in Bs]
                for (t, An2), (_, Bn0) in zip(Ans, Bns):
                    C = conv_one(img, t, "s")
                    combine(img, t, An2, Bn0, C)

            # tile t needs input rows <= 4t+4; band b supplies rows < 28(b+1).
            # Binarize lands in 14-row strips, interleaved BETWEEN tiles so
            # the long Sign ops never head-of-line-block the short PSUM
            # evacuations in the static ScalarE queue; strips stay three
            # tile-groups ahead of their consumers.
            ranges = [range(0, 6), range(6, 13), range(13, 20), range(20, 28)]
            groups = [(i, b) for i in range(IMGS) for b in range(H // BAND)]
            # head: first strip's DMAs go out on ScalarE's own ring (ready
            # ~1.5us before Sync) so they don't queue behind the weight
            # DMAs; ScalarE then alternates strip/weight signs to shorten
            # the critical chain
            st_r = stage_dma(0, 0, 0, 14, dma_eng=nc.scalar)
            st_i = stage_dma(0, 1, 0, 14, dma_eng=nc.scalar)
            nc.scalar.activation(wq_r[:], wr_v, AF.Sign, bias=eps_pos[:], scale=1.0)
            sign_stage(0, 0, 0, 14, st_r)
            nc.scalar.activation(wq_i[:], wi_v, AF.Sign, bias=eps_pos[:], scale=1.0)
            sign_stage(0, 1, 0, 14, st_i)
            nc.vector.tensor_tensor(wq_s[:], wq_r[:], wq_i[:], op=ALU.add)
            sum_rows(0, 0, 14)
            # All later strips are signed JUST IN TIME inside the conv loop
            # (one band ahead of their consumers): ScalarE's FIFO must never
            # hold more than ~2 long Sign ops ahead of pending PSUM
            # evacuations, or the banks clog and the PE stalls.
            strip_plan = ([(0, r) for r in range(14, H, 14)]
                          + [(1, r) for r in range(0, H, 14)] + [None, None])
            si = 0
            for gi, (img, b) in enumerate(groups):
                tiles = list(ranges[b])
                if gi == 0:
                    # 3 kind-major tiles give the i/s inputs ~3us of runway;
                    # the (0,14) strip signs land between the A and B phases
                    # so tile 3's s-conv finds its rows summed in time
                    conv_tiles_kindmajor(img, tiles[:3],
                                         mid=lambda: binarize_rows(0, 14, 14))
                    si += 1
                else:
                    conv_tiles(img, tiles[:2])
                if strip_plan[si]:
                    binarize_rows(strip_plan[si][0], strip_plan[si][1], 14)
                si += 1
                conv_tiles(img, tiles[2:4] if gi else tiles[3:4])
                if strip_plan[si]:
                    binarize_rows(strip_plan[si][0], strip_plan[si][1], 14)
                si += 1
                conv_tiles(img, tiles[4:], last=(gi == len(groups) - 1))

    _split_multiwait(nc)
    return nc


def _prep(x, weight_real, weight_imag, bias):
    import ml_dtypes

    # bf16 upload: halves HBM traffic; sign(x + 1e-6) flips only for
    # |x| ~< 1e-6 * ulp, i.e. ~20 elements across the whole batch.
    x = np.ascontiguousarray(np.asarray(x, dtype=np.float32).astype(ml_dtypes.bfloat16))
    wr = np.asarray(weight_real, dtype=np.float32).astype(ml_dtypes.bfloat16)
    wi = np.asarray(weight_imag, dtype=np.float32).astype(ml_dtypes.bfloat16)
    bias = np.asarray(bias, dtype=np.float32)
    wrT = np.ascontiguousarray(wr.transpose(1, 2, 3, 0).reshape(CPB, 9 * CPB))
    wiT = np.ascontiguousarray(wi.transpose(1, 2, 3, 0).reshape(CPB, 9 * CPB))
    bias2 = np.ascontiguousarray(bias.reshape(2, CPB).T)
    return [
        {"x": x[IMGS * c:IMGS * (c + 1)], "wrT": wrT, "wiT": wiT, "bias2": bias2}
        for c in range(N_CORES)
    ]


def kernel(x, weight_real, weight_imag, bias):
    in_maps = _prep(x, weight_real, weight_imag, bias)
    nc = build_nc()
    res = run_bass_kernel_spmd(nc, in_maps, core_ids=list(range(N_CORES)))
    out = np.concatenate([res.results[i]["out"] for i in range(N_CORES)], axis=0)
    return out.astype(np.float32)


def run_traced(x, weight_real, weight_imag, bias, **trace_kwargs):
    """test.py entry: same as kernel() but with neuron-profile tracing."""
    in_maps = _prep(x, weight_real, weight_imag, bias)
    nc = build_nc()
    res = run_bass_kernel_spmd(
        nc, in_maps, core_ids=list(range(N_CORES)), trace=True, **trace_kwargs
    )
    out = np.concatenate([res.results[i]["out"] for i in range(N_CORES)], axis=0)
    return out.astype(np.float32), res



# revision 51
# speedup vs baseline: 1.0070x; 1.0070x over previous
"""Binary complex conv (BC conv) on 8 TRN2 NeuronCores.

Reference computation:
    xb = sign(x + 1e-6)                      # (16, 256, 112, 112)
    wr = sign(weight_real + 1e-6)            # (128, 128, 3, 3)
    wi = sign(weight_imag + 1e-6)
    kernel = [[wr, -wi], [wi, wr]]           # (256, 256, 3, 3)
    out = conv2d(xb, kernel, pad=1) + bias   # (16, 256, 112, 112)

Strategy: pure data-parallel over batch (2 images per core); everything
else on-device, numerically exact in the matmuls (all operands are
+-1/0/+-2 -> exact in fp8e4; PSUM accumulates fp32; x is uploaded and
out downloaded as bf16, within the 2e-2 gate by a wide margin).

Tricks on top of the direct conv:
 * Karatsuba for the complex structure: A = xr*wr, B = xi*wi,
   C = (xr+xi)*(wr+wi); out_real = A-B, out_imag = C-A-B.
   3 convs of 128 input channels instead of 4.
 * fp8 DoubleRow: each binarized frame is stored with row stride 114;
   conv taps in raster order have flat offsets [-115,-114,-113,-1,0,1,
   113,114,115], so consecutive taps pair into DoubleRow matmuls
   (contraction 256) with pair strides 1/112/1/1 + one normal matmul.
 * 448-wide matmuls: the moving operand walks [pair, row(x114), col(x112)]
   so the 2 pad columns per frame row are never streamed and the
   4-output-row PSUM bank is compact [128, 4, 112].
 * bf16 I/O halves HBM traffic (52.6 -> 26.9 MB/core): no output-DMA
   backlog at the tail, no DMA-contention stalls mid-stream.

Schedule (measured on HW): PE streams 840 matmuls back-to-back at
1 col/cycle with LDWEIGHTS hidden (~164us core at 2.36GHz, zero stalls).
Head ~15us: engine preamble (~7us, fixed) + junk-MM HAM warmup sized to
end exactly at data-ready; the Sign ACT_TABLE_LOAD is hoisted by a dummy
activation; wr/strips ride separate DMA rings.  Strips are binarized
JUST IN TIME between conv tile-pairs -- ScalarE's FIFO must never hold
long Sign ops ahead of pending PSUM evacuations or the banks clog and
the PE stalls.  Tail ~6us: last tile runs its s-conv first and splits
evac/combine/DMA into 2-row halves.
"""

import numpy as np

import concourse.bass as bass
import concourse.tile as tile
from concourse import mybir
from concourse.bass_utils import run_bass_kernel_spmd

N_CORES = 8
B = 16
CPB = 128          # channels per block (partition dim)
H = W = 112
RS = 114           # frame row stride
FROWS = 116        # 114 padded rows + 2 junk margin rows
IMGS = 2
TROWS = 4          # output rows per matmul tile
NT = TROWS * RS    # matmul free dim (456)
NTILES = H // TROWS
BAND = 28          # input rows binarized per activation op
EPS = 1e-6

F32 = mybir.dt.float32
BF16 = mybir.dt.bfloat16
FP8 = mybir.dt.float8e4
AF = mybir.ActivationFunctionType
DRM = mybir.MatmulPerfMode.DoubleRow
ALU = mybir.AluOpType

# tap flat offsets in raster order; pairs (0,1) (2,3) (4,5) (6,7), single 8
TAP_OFF = [dy * RS + dx for dy in (-1, 0, 1) for dx in (-1, 0, 1)]


def _split_multiwait(nc):
    """Walrus in this container rejects >1 semaphore wait per instruction
    ("Too many sync wait commands"); hoist extra waits onto preceding nops
    on the same engine."""
    import bass_rust

    for f in nc.m.functions:
        for bb in f.blocks:
            new_insts = []
            for inst in bb.instructions:
                si = inst.sync_info
                waits = list(si.on_wait) if si is not None and si.on_wait else []
                if len(waits) > 1:
                    for w in waits[:-1]:
                        nop = mybir.InstNoOp(
                            name=nc.get_next_instruction_name(),
                            engine=inst.engine,
                            ins=[],
                            outs=[],
                        )
                        nop.sync_info = bass_rust.SyncInfo(on_wait=[w], on_update=[])
                        new_insts.append(nop)
                    si.on_wait = [waits[-1]]
                    inst.sync_info = si
                new_insts.append(inst)
            bb.instructions = new_insts


def build_nc():
    nc = bass.Bass()

    x_ext = nc.declare_dram_parameter("x", [IMGS, 2 * CPB, H, W], BF16, isOutput=False)
    wr_ext = nc.declare_dram_parameter("wrT", [CPB, 9 * CPB], BF16, isOutput=False)
    wi_ext = nc.declare_dram_parameter("wiT", [CPB, 9 * CPB], BF16, isOutput=False)
    bias_ext = nc.declare_dram_parameter("bias2", [CPB, 2], F32, isOutput=False)
    out_ext = nc.declare_dram_parameter("out", [IMGS, 2 * CPB, H, W], BF16, isOutput=True)

    x_flat = x_ext.rearrange("b c h w -> (b c) h w")       # [512, 112, 112]
    out_flat = out_ext.rearrange("b c h w -> (b c) h w")

    with tile.TileContext(nc) as tc:
        with (
            tc.tile_pool(name="wstage", bufs=2) as wstage_pool,
            tc.tile_pool(name="wbin", bufs=1) as wbin_pool,
            tc.tile_pool(name="biasp", bufs=1) as bias_pool,
            tc.tile_pool(name="xq", bufs=1) as xq_pool,
            tc.tile_pool(name="stage", bufs=4) as stage_pool,
            tc.tile_pool(name="tmp", bufs=6) as tmp_pool,
            tc.tile_pool(name="outsb", bufs=8) as out_pool,
            tc.tile_pool(name="psum", bufs=8, space="PSUM") as psum_pool,
        ):
            # HAM warmup: dummy matmuls on junk data with no dependencies so
            # the PE clock-gate reaches 8/8 before the first real matmul.
            # The memset is VectorE's very first op; sized to end right as
            # the first binarized strip + weights land.
            junk = bias_pool.tile([CPB, 512], FP8, tag="junk")
            nc.vector.memset(junk[:], 1.0)
            jps = psum_pool.tile([CPB, 512], F32, tag="ps", name="jps")
            for _ in range(16):
                nc.tensor.matmul(jps[:], junk[:, :CPB], junk[:], start=True,
                                 stop=True)

            # per-partition scalar constant for activation bias
            eps_pos = bias_pool.tile([CPB, 1], F32, tag="epsp")
            nc.vector.memset(eps_pos[:], EPS)

            # dummy Sign on ScalarE: hoists the ~1.3us ACT_TABLE_LOAD off
            # the first real sign's critical path
            tbl = bias_pool.tile([CPB, 1], FP8, tag="tbl")
            nc.scalar.activation(tbl[:], eps_pos[:], AF.Sign, bias=eps_pos[:],
                                 scale=1.0)

            # ---- weights: wr first on Sync's ring (wr -> wq_r sign gates
            # the first conv); the head x strips ride ScalarE's ring ----
            wr_f32 = wstage_pool.tile([CPB, 9 * CPB], BF16, tag="wstage")
            nc.sync.dma_start(wr_f32[:], wr_ext[:])
            wi_f32 = wstage_pool.tile([CPB, 9 * CPB], BF16, tag="wstage")
            nc.sync.dma_start(wi_f32[:, :576], wi_ext[:, :576])
            nc.sync.dma_start(wi_f32[:, 576:], wi_ext[:, 576:])

            # binarized fp8 weights [ci, tap, co]; wq_s = wq_r + wq_i
            wq_r = wbin_pool.tile([CPB, 9, CPB], FP8, tag="wqr")
            wq_i = wbin_pool.tile([CPB, 9, CPB], FP8, tag="wqi")
            wq_s = wbin_pool.tile([CPB, 9, CPB], FP8, tag="wqs")
            wr_v = wr_f32[:].rearrange("p (t c) -> p t c", c=CPB)
            wi_v = wi_f32[:].rearrange("p (t c) -> p t c", c=CPB)

            bias_sb = bias_pool.tile([CPB, 2], F32)
            nc.gpsimd.dma_start(bias_sb[:], bias_ext[:])
            bias_ir = bias_pool.tile([CPB, 1], F32, tag="biasir")
            nc.vector.tensor_add(bias_ir[:], bias_sb[:, 1:2], bias_sb[:, 0:1])

            # ---- persistent binarized fp8 frames ----
            # frame: [128, FROWS, RS]; frame row = padded row + 1 (1 junk
            # margin row on top); cols 0 / 113 are the zero pad columns,
            # cols 114-115 slack (only ever read into discarded pad lanes)
            def frame(nm):
                return xq_pool.tile([CPB, FROWS, RS], FP8, tag=nm, name=nm)

            xqr = [frame(f"xqr{i}") for i in range(IMGS)]
            xqi = [frame(f"xqi{i}") for i in range(IMGS)]
            xqs = [frame(f"xqs{i}") for i in range(IMGS)]
            for i in range(IMGS):
                eng = nc.vector if i == 0 else nc.gpsimd
                for t in (xqr[i], xqi[i], xqs[i]):
                    eng.memset(t[:, 1:2, :], 0.0)          # padded row 0
                    eng.memset(t[:, 114:115, :], 0.0)      # padded row 113
                    eng.memset(t[:, 1:115, 0:1], 0.0)      # padded col 0
                    eng.memset(t[:, 1:115, 113:114], 0.0)  # padded col 113

            flat = {}
            for i in range(IMGS):
                flat[("r", i)] = xqr[i][:].rearrange("p r c -> p (r c)")
                flat[("i", i)] = xqi[i][:].rearrange("p r c -> p (r c)")
                flat[("s", i)] = xqs[i][:].rearrange("p r c -> p (r c)")

            # ---- binarize input + build the sum frame, band by band ----
            def stage_dma(img, cib, r0, nr, dma_eng=None):
                ch0 = img * 2 * CPB + cib * CPB
                st = stage_pool.tile([CPB, BAND, W], BF16, tag="stage")
                (dma_eng or nc.sync).dma_start(
                    st[:, :nr, :],
                    x_flat[ch0:ch0 + CPB, r0:r0 + nr, :],
                )
                return st

            def sign_stage(img, cib, r0, nr, st):
                rows = slice(r0 + 2, r0 + 2 + nr)
                dst = xqr if cib == 0 else xqi
                nc.scalar.activation(
                    dst[img][:, rows, 1:113], st[:, :nr, :],
                    AF.Sign, bias=eps_pos[:], scale=1.0,
                )

            def stage_sign(img, cib, r0, nr, dma_eng=None):
                sign_stage(img, cib, r0, nr, stage_dma(img, cib, r0, nr, dma_eng))

            def sum_rows(img, r0, nr):
                rows = slice(r0 + 2, r0 + 2 + nr)
                nc.vector.tensor_tensor(
                    xqs[img][:, rows, 1:113],
                    xqr[img][:, rows, 1:113],
                    xqi[img][:, rows, 1:113],
                    op=ALU.add,
                )

            def binarize_rows(img, r0, nr):
                stage_sign(img, 0, r0, nr)
                stage_sign(img, 1, r0, nr)
                sum_rows(img, r0, nr)

            def binarize_band(img, b, strips=1):
                r0 = b * BAND
                step = BAND // strips
                for s in range(strips):
                    binarize_rows(img, r0 + s * step, step)

            def conv_one(img, t, kind):
                # free dim walks [row, col] with the frame's 2 pad columns
                # skipped: 448 columns per matmul instead of 456 (-1.75%
                # PE time), and the PSUM bank comes out compact
                base = (4 * t + 2) * RS + 1
                w3 = {"r": wq_r, "i": wq_i, "s": wq_s}[kind]
                xf = flat[(kind, img)]
                ps = psum_pool.tile([CPB, TROWS, W], F32, tag="ps",
                                    name=f"ps_{kind}{img}_{t}")
                part = [list(xf.ap)[0][0], CPB]
                for p in range(4):
                    o0, o1 = TAP_OFF[2 * p], TAP_OFF[2 * p + 1]
                    rhs = bass.AP(
                        xf.tensor, xf.offset + o0 + base,
                        [part, [o1 - o0, 2], [RS, TROWS], [1, W]],
                    )
                    nc.tensor.matmul(
                        ps[:], w3[:, 2 * p:2 * p + 2, :], rhs,
                        start=(p == 0), stop=False, perf_mode=DRM,
                    )
                rhs8 = bass.AP(
                    xf.tensor, xf.offset + TAP_OFF[8] + base,
                    [part, [RS, TROWS], [1, W]],
                )
                nc.tensor.matmul(ps[:], w3[:, 8, :], rhs8,
                                 start=False, stop=True)
                return ps

            # out_real = A - B + bias_r ; out_imag = C - A - B + bias_i
            # ScalarE (fast PSUM port) evacuates each bank compactly
            # right after its conv, exactly one reader per bank:
            #   An2 = A + bias_r ; Bn0 = -B
            # then out_real = An2 + Bn0 (SBUF-only, GpSimd)
            #      out_imag = ((C - An2) + (bias_i+bias_r)) + Bn0
            def evac_A(A):
                An2 = tmp_pool.tile([CPB, TROWS, W], F32, tag="An")
                nc.scalar.activation(An2[:], A[:], AF.Identity,
                                     bias=bias_sb[:, 0:1], scale=1.0)
                return An2

            def evac_B(Bp):
                Bn0 = tmp_pool.tile([CPB, TROWS, W], F32, tag="Bn")
                nc.scalar.activation(Bn0[:], Bp[:], AF.Identity,
                                     bias=0.0, scale=-1.0)
                return Bn0

            def combine(img, t, An2, Bn0, C, re_eng=None):
                osb = out_pool.tile([CPB, 2, TROWS, W], BF16, tag="osb")
                t5 = tmp_pool.tile([CPB, TROWS, W], F32, tag="t5")
                nc.vector.tensor_sub(t5[:], C[:], An2[:])
                (re_eng or nc.gpsimd).tensor_tensor(
                    osb[:, 0], An2[:], Bn0[:], op=ALU.add)
                nc.vector.scalar_tensor_tensor(
                    osb[:, 1], t5[:], bias_ir[:], Bn0[:],
                    op0=ALU.add, op1=ALU.add,
                )

                # one DMA for both channel halves: dst walks [ch-within-
                # block, block, row, col] to match the tile's layout
                dst = bass.AP(
                    out_flat.tensor,
                    img * 2 * CPB * H * W + 4 * t * W,
                    [[H * W, CPB], [CPB * H * W, 2], [W, TROWS], [1, W]],
                )
                nc.sync.dma_start(dst, osb[:])

            def combine_half(img, t, An2, Bn0h, C, h):
                # 2-row half of the last tile: short serial chain after the
                # final matmul, so the closing DMA completes sooner
                rows = slice(2 * h, 2 * h + 2)
                osb = out_pool.tile([CPB, 2, 2, W], BF16, tag="osbh")
                t5 = tmp_pool.tile([CPB, 2, W], F32, tag="t5h")
                nc.vector.tensor_sub(t5[:], C[:, rows], An2[:, rows])
                nc.vector.tensor_tensor(
                    osb[:, 0], An2[:, rows], Bn0h[:], op=ALU.add)
                nc.vector.scalar_tensor_tensor(
                    osb[:, 1], t5[:], bias_ir[:], Bn0h[:],
                    op0=ALU.add, op1=ALU.add,
                )
                dst = bass.AP(
                    out_flat.tensor,
                    img * 2 * CPB * H * W + (4 * t + 2 * h) * W,
                    [[H * W, CPB], [CPB * H * W, 2], [W, 2], [1, W]],
                )
                # halves go out on different rings so their ~0.65us issue
                # slices run in parallel at the very end
                (nc.scalar if h else nc.sync).dma_start(dst, osb[:])

            def evac_B_half(Bp, h):
                Bn0 = tmp_pool.tile([CPB, 2, W], F32, tag="Bnh")
                nc.scalar.activation(Bn0[:], Bp[:, 2 * h:2 * h + 2],
                                     AF.Identity, bias=0.0, scale=-1.0)
                return Bn0

            def conv_tiles(img, tiles, last=False):
                for t in tiles:
                    if last and t == tiles[-1]:
                        # last tile: s-conv first so t5 is off the critical
                        # path; evac+combine+DMA split into 2-row halves
                        C = conv_one(img, t, "s")
                        An2 = evac_A(conv_one(img, t, "r"))
                        Bp = conv_one(img, t, "i")
                        b0 = evac_B_half(Bp, 0)
                        combine_half(img, t, An2, b0, C, 0)
                        b1 = evac_B_half(Bp, 1)
                        combine_half(img, t, An2, b1, C, 1)
                        continue
                    An2 = evac_A(conv_one(img, t, "r"))
                    Bn0 = evac_B(conv_one(img, t, "i"))
                    C = conv_one(img, t, "s")
                    combine(img, t, An2, Bn0, C)

            def conv_tiles_kindmajor(img, tiles, mid=None):
                # first tiles: group by kind so the i/s convs' inputs
                # (wq_i, wq_s, xqs) get extra time to become ready
                As = [(t, conv_one(img, t, "r")) for t in tiles]
                Ans = [(t, evac_A(A)) for t, A in As]
                if mid:
                    mid()
                Bs = [(t, conv_one(img, t, "i")) for t in tiles]
                Bns = [(t, evac_B(B)) for t, B in Bs]
                for (t, An2), (_, Bn0) in zip(Ans, Bns):
                    C = conv_one(img, t, "s")
                    combine(img, t, An2, Bn0, C)

            # tile t needs input rows <= 4t+4; band b supplies rows < 28(b+1).
            # Binarize lands in 14-row strips, interleaved BETWEEN tiles so
            # the long Sign ops never head-of-line-block the short PSUM
            # evacuations in the static ScalarE queue; strips stay three
            # tile-groups ahead of their consumers.
            ranges = [range(0, 6), range(6, 13), range(13, 20), range(20, 28)]
            groups = [(i, b) for i in range(IMGS) for b in range(H // BAND)]
            # head: first strip's DMAs go out on ScalarE's own ring (ready
            # ~1.5us before Sync) so they don't queue behind the weight
            # DMAs; ScalarE then alternates strip/weight signs to shorten
            # the critical chain
            stage_sign(0, 0, 0, 14, dma_eng=nc.scalar)
            nc.scalar.activation(wq_r[:], wr_v, AF.Sign, bias=eps_pos[:], scale=1.0)
            stage_sign(0, 1, 0, 14, dma_eng=nc.scalar)
            nc.scalar.activation(wq_i[:], wi_v, AF.Sign, bias=eps_pos[:], scale=1.0)
            nc.vector.tensor_tensor(wq_s[:], wq_r[:], wq_i[:], op=ALU.add)
            sum_rows(0, 0, 14)
            # All later strips are signed JUST IN TIME inside the conv loop
            # (one band ahead of their consumers): ScalarE's FIFO must never
            # hold more than ~2 long Sign ops ahead of pending PSUM
            # evacuations, or the banks clog and the PE stalls.
            strip_plan = ([(0, r) for r in range(14, H, 14)]
                          + [(1, r) for r in range(0, H, 14)] + [None])
            si = 0
            for gi, (img, b) in enumerate(groups):
                tiles = list(ranges[b])
                if gi == 0:
                    conv_tiles_kindmajor(img, tiles[:2])
                else:
                    conv_tiles(img, tiles[:2])
                if strip_plan[si]:
                    binarize_rows(strip_plan[si][0], strip_plan[si][1], 14)
                si += 1
                conv_tiles(img, tiles[2:4])
                if strip_plan[si]:
                    binarize_rows(strip_plan[si][0], strip_plan[si][1], 14)
                si += 1
                conv_tiles(img, tiles[4:], last=(gi == len(groups) - 1))

    _split_multiwait(nc)
    return nc


def _prep(x, weight_real, weight_imag, bias):
    import ml_dtypes

    # bf16 upload: halves HBM traffic; sign(x + 1e-6) flips only for
    # |x| ~< 1e-6 * ulp, i.e. ~20 elements across the whole batch.
    x = np.ascontiguousarray(np.asarray(x, dtype=np.float32).astype(ml_dtypes.bfloat16))
    wr = np.asarray(weight_real, dtype=np.float32).astype(ml_dtypes.bfloat16)
    wi = np.asarray(weight_imag, dtype=np.float32).astype(ml_dtypes.bfloat16)
    bias = np.asarray(bias, dtype=np.float32)
    wrT = np.ascontiguousarray(wr.transpose(1, 2, 3, 0).reshape(CPB, 9 * CPB))
    wiT = np.ascontiguousarray(wi.transpose(1, 2, 3, 0).reshape(CPB, 9 * CPB))
    bias2 = np.ascontiguousarray(bias.reshape(2, CPB).T)
    return [
        {"x": x[IMGS * c:IMGS * (c + 1)], "wrT": wrT, "wiT": wiT, "bias2": bias2}
        for c in range(N_CORES)
    ]


def kernel(x, weight_real, weight_imag, bias):
    in_maps = _prep(x, weight_real, weight_imag, bias)
    nc = build_nc()
    res = run_bass_kernel_spmd(nc, in_maps, core_ids=list(range(N_CORES)))
    out = np.concatenate([res.results[i]["out"] for i in range(N_CORES)], axis=0)
    return out.astype(np.float32)


def run_traced(x, weight_real, weight_imag, bias, **trace_kwargs):
    """test.py entry: same as kernel() but with neuron-profile tracing."""
    in_maps = _prep(x, weight_real, weight_imag, bias)
    nc = build_nc()
    res = run_bass_kernel_spmd(
        nc, in_maps, core_ids=list(range(N_CORES)), trace=True, **trace_kwargs
    )
    out = np.concatenate([res.results[i]["out"] for i in range(N_CORES)], axis=0)
    return out.astype(np.float32), res

